# revision 12
# baseline (speedup 1.0000x reference)
"""Trainium2 Bass kernel for nn_MoE_32332513804634.

MoE: 16 routed experts (top-6, softmax-then-bias routing) + dense shared
expert, T=4096 tokens, D=2048, H=1408, HS=2816, fp32.

Strategy (8 NeuronCores, SPMD):
  - Host computes the gate (cheap) and per-expert token lists.
  - Expert parallelism as a flat list of variable-width "jobs" (128-token
    granularity, up to 1024 tokens each). All cores run the identical
    job-width schedule; the host binds each (core, job) to an arbitrary
    expert piece, so load balance is a pure host-side packing problem.
    A small schedule search minimizes padded capacity (~3200 tokens/core
    vs the 3072 ideal).
  - Weights are streamed per job in DMA-friendly pre-tiled DRAM layouts
    (per-partition contiguous runs of 8-22KB), loaded once per job and
    reused across the job's 512-token chunks.
  - Shared expert is token-parallel: each core runs its 512 tokens
    through the full HS=2816 hidden dim (22 even 128-subtiles, no pad).
  - bf16 matmuls accumulate in fp32 PSUM; per-token combine weights are
    applied as a per-partition DVE scale on the PSUM->SBUF copy; the
    second-layer biases (cw*b2, bs2) are added on the host.
"""

import sys
import numpy as np

sys.path.insert(0, "/opt/trn_rl_repo")

import concourse.bass as bass  # noqa: E402
import concourse.tile as tile  # noqa: E402
from concourse import bacc, mybir  # noqa: E402
from concourse.bass_utils import run_bass_kernel_spmd  # noqa: E402

T = 4096
D = 2048
H = 1408
E = 16
TOP_K = 6
HS = 2816
N_CORES = 8
HM = H // 128           # 11
KO = D // 128           # 16
HMS = HS // 128         # 22
TS = T // N_CORES       # 512 shared-expert tokens per core
F32 = mybir.dt.float32
BF16 = mybir.dt.bfloat16
MM_DT = BF16

_PROGRAM_CACHE: dict = {}


def _to_mm(a):
    import ml_dtypes
    return np.ascontiguousarray(a).astype(ml_dtypes.bfloat16)


def _host_gate(xf, gate_w, gate_b):
    """Numpy replica of the reference gate. Returns cw [T, E] dense combine
    weights and per-expert token lists (ascending)."""
    scores = xf @ gate_w.T
    m = scores.max(axis=-1, keepdims=True)
    p = np.exp(scores - m, dtype=np.float32)
    probs = p / p.sum(axis=-1, keepdims=True)
    biased = probs + gate_b
    idx = np.argpartition(biased, E - TOP_K, axis=1)[:, E - TOP_K:]
    mask = np.zeros((xf.shape[0], E), dtype=bool)
    mask[np.arange(xf.shape[0])[:, None], idx] = True
    cw = np.where(mask, probs, 0.0).astype(np.float32)
    toks = [np.flatnonzero(mask[:, e]).astype(np.int64) for e in range(E)]
    return cw, toks


def _group_partition(counts, widths):
    """Partition the 8x copies of `widths` cells into one cell-group per
    expert with group capacity >= count (global slack bounded by the
    schedule's total padding). Returns groups[e] = list of cell widths,
    or None. Backtracking over experts in descending count order."""
    distinct = sorted(set(widths), reverse=True)
    avail = {w: 8 * widths.count(w) for w in distinct}
    total_cap = sum(w * n for w, n in avail.items())
    order = np.argsort(-np.asarray(counts))
    slack = total_cap - int(np.sum(counts))
    if slack < 0:
        return None

    def combos(target, limit):
        """Cell-width multisets (from current avail) with sum in
        [target, target+limit], cheapest (least waste, fewest) first."""
        out = []

        def rec(i, acc, cap):
            if target <= cap <= target + limit:
                out.append(list(acc))
                return  # adding more cells only wastes
            if i >= len(distinct) or len(out) >= 400:
                return
            w = distinct[i]
            max_n = min(avail[w], -(-(target - cap) // w))
            for n in range(max_n, -1, -1):
                if cap + n * w > target + limit:
                    continue
                acc.extend([w] * n)
                rec(i + 1, acc, cap + n * w)
                if n:
                    del acc[-n:]

        rec(0, [], 0)
        out.sort(key=lambda g: (sum(g), len(g)))
        return out[:80]

    groups = [None] * E
    budget = [4000]

    def solve(k, slack_left):
        if k == len(order):
            return True
        if budget[0] <= 0:
            return False
        budget[0] -= 1
        e = int(order[k])
        target = int(counts[e])
        for g in combos(target, slack_left):
            for w in g:
                avail[w] -= 1
            groups[e] = g
            if solve(k + 1, slack_left - (sum(g) - target)):
                return True
            for w in g:
                avail[w] += 1
            groups[e] = None
        return False

    if not solve(0, slack):
        return None
    return groups


def _pack(counts, widths):
    """Assign expert pieces to an 8 x len(widths) grid of cells via the
    group-partition solver. Returns assignment[core][j] = (expert, start,
    n) | None, or None if infeasible."""
    groups = _group_partition(counts, widths)
    if groups is None:
        return None
    # free cells per width: list of (core, j)
    free = {}
    for j, w in enumerate(widths):
        for c in range(N_CORES):
            free.setdefault(w, []).append((c, j))
    asg = [[None] * len(widths) for _ in range(N_CORES)]
    for e in range(E):
        g = sorted(groups[e], reverse=True)
        rem = int(counts[e])
        start = 0
        for w in g:
            c, j = free[w].pop()
            n = min(rem, w)
            if n > 0:
                asg[c][j] = (int(e), start, int(n))
            start += n
            rem -= n
        assert rem == 0
    return asg


def _plan_jobs(counts):
    """Pick the per-core job-width schedule (identical across cores) and
    the piece assignment. Minimizes padded token capacity, then job
    count, preferring wider minimum job width (DMA-friendlier)."""
    total = int(np.sum(counts))
    percore_lo = -(-total // N_CORES)
    wvocab = [1024, 896, 768, 640, 512, 384, 256, 128]
    cands = []

    def gen(i, acc, cap, ndist):
        if percore_lo <= cap <= percore_lo + 640 and acc:
            cands.append((cap, len(acc), -min(acc), tuple(acc)))
        if i >= len(wvocab) or cap > percore_lo + 640 or len(acc) >= 9:
            return
        w = wvocab[i]
        for n in range(0, min(8, (percore_lo + 640 - cap) // w) + 1):
            nd = ndist + (1 if n else 0)
            if nd > 3:
                break
            gen(i + 1, acc + [w] * n, cap + n * w, nd)

    gen(0, [], 0, 0)
    cands = sorted(set(cands))
    # second pass with more generous capacity in case nothing tight packs
    loose = [(cap + 8 * 512, njobs + 1, mn, w + (512,))
             for cap, njobs, mn, w in cands]
    for cap, njobs, _, widths in cands + loose:
        asg = _pack(counts, widths)
        if asg is not None:
            order = np.argsort([w for w in widths], kind="stable")[::-1]
            widths_o = tuple(int(widths[i]) for i in order)
            asg_o = [[asg[c][int(i)] for i in order] for c in range(N_CORES)]
            return widths_o, asg_o
    raise RuntimeError("no feasible job schedule found")


def _build_program(widths):
    """Build the SPMD Bass program for the given per-core job widths."""
    nc = bacc.Bacc("TRN2", debug=False, num_devices=N_CORES)

    ins = {}
    outs = {}

    def din(name, shape, dt=MM_DT):
        ins[name] = nc.dram_tensor(name, list(shape), dt, kind="ExternalInput").ap()
        return ins[name]

    def dout(name, shape, dt=F32):
        outs[name] = nc.dram_tensor(name, list(shape), dt, kind="ExternalOutput").ap()
        return outs[name]

    for j, w in enumerate(widths):
        din(f"xg{j}", (128, KO, w))
        din(f"w13_{j}", (HM * 128, KO, 256))
        din(f"w2_{j}", (4 * 128, HM, 512))
        din(f"b13_{j}", (128, 2 * HM), F32)
        din(f"scl{j}", (128, w // 128), F32)
        dout(f"oe{j}", (w, D))
    din("xs", (128, KO, TS))
    din("ws13", (HMS * 128, KO, 256))
    din("ws2", (4 * 128, HMS, 512))
    din("bs13", (128, 2 * HMS), F32)
    dout("zs", (TS, D))

    with tile.TileContext(nc) as tc:
        with (
            tc.tile_pool(name="xpool", bufs=2) as xpool,
            tc.tile_pool(name="hpool", bufs=2) as hpool,
            tc.tile_pool(name="wcol", bufs=2) as wcol,
            tc.tile_pool(name="w2pool", bufs=2) as w2pool,
            tc.tile_pool(name="tmp", bufs=2) as tmp,
            tc.tile_pool(name="opool", bufs=3) as opool,
            tc.tile_pool(name="cpool", bufs=1) as cpool,
            tc.tile_pool(name="pp", bufs=2, space="PSUM") as pp,
        ):
            def mlp_job(xg_ap, w13_ap, w2_ap, b13_ap, scl_ap, out_ap,
                        w, n_hm, tag, scale_one):
                """One job: out = scale * (swiglu(x) @ W2^T), W2 bias on host."""
                b13sb = cpool.tile([128, 2 * n_hm], F32, tag=f"b{tag}")
                nc.sync.dma_start(b13sb[:], b13_ap)
                if not scale_one:
                    sclsb = cpool.tile([128, w // 128], F32, tag=f"s{tag}")
                    nc.sync.dma_start(sclsb[:], scl_ap)
                chunks = []
                o = 0
                while o < w:
                    cw_ = min(512, w - o)
                    chunks.append((o, cw_))
                    o += cw_

                xsb = xpool.tile([128, KO, w], MM_DT, tag="xg")
                # two-half load: first matmuls (ko<8) only wait on half the x
                nc.sync.dma_start(xsb[:, 0:KO // 2, :], xg_ap[:, 0:KO // 2, :])
                nc.sync.dma_start(xsb[:, KO // 2:, :], xg_ap[:, KO // 2:, :])
                hsb = hpool.tile([128, n_hm, w], MM_DT, tag="h")
                for hm in range(n_hm):
                    wsb = wcol.tile([128, KO, 256], MM_DT, tag="w13")
                    nc.sync.dma_start(wsb[:], w13_ap[hm * 128:(hm + 1) * 128])
                    for (c0, cw_) in chunks:
                        ps1 = pp.tile([128, cw_], F32, tag="ph1")
                        for ko in range(KO):
                            nc.tensor.matmul(ps1[:], wsb[:, ko, 0:128],
                                             xsb[:, ko, c0:c0 + cw_],
                                             start=(ko == 0), stop=(ko == KO - 1))
                        ps3 = pp.tile([128, cw_], F32, tag="ph3")
                        for ko in range(KO):
                            nc.tensor.matmul(ps3[:], wsb[:, ko, 128:256],
                                             xsb[:, ko, c0:c0 + cw_],
                                             start=(ko == 0), stop=(ko == KO - 1))
                        h1t = tmp.tile([128, cw_], F32, tag="h1t")
                        nc.scalar.activation(h1t[:], ps1[:],
                                             mybir.ActivationFunctionType.Silu,
                                             bias=b13sb[:, hm:hm + 1])
                        h3t = tmp.tile([128, cw_], F32, tag="h3t")
                        nc.scalar.activation(h3t[:], ps3[:],
                                             mybir.ActivationFunctionType.Identity,
                                             bias=b13sb[:, n_hm + hm:n_hm + hm + 1])
                        nc.vector.tensor_mul(hsb[:, hm, c0:c0 + cw_],
                                             h1t[:], h3t[:])
                for dm in range(4):
                    w2sb = w2pool.tile([128, n_hm, 512], MM_DT, tag="w2s")
                    nc.sync.dma_start(w2sb[:], w2_ap[dm * 128:(dm + 1) * 128])
                    for tch in range(w // 128):
                        ps2 = pp.tile([128, 512], F32, tag="po", bufs=4)
                        for k in range(n_hm):
                            nc.tensor.matmul(ps2[:],
                                             hsb[:, k, tch * 128:(tch + 1) * 128],
                                             w2sb[:, k, :],
                                             start=(k == 0), stop=(k == n_hm - 1))
                        osb = opool.tile([128, 512], F32, tag="osb")
                        if scale_one:
                            nc.vector.tensor_copy(osb[:], ps2[:])
                        else:
                            nc.vector.tensor_scalar_mul(
                                osb[:], ps2[:], sclsb[:, tch:tch + 1])
                        nc.sync.dma_start(
                            out_ap[tch * 128:(tch + 1) * 128,
                                   dm * 512:(dm + 1) * 512],
                            osb[:])

            # order: smallest routed job first (short lead-in), then the
            # shared expert (its heavy weight stream hides under 225us of
            # compute), then the remaining routed jobs, largest first
            emit = [len(widths) - 1, "sh"] + list(range(len(widths) - 1))
            for j in emit:
                if j == "sh":
                    mlp_job(ins["xs"], ins["ws13"], ins["ws2"], ins["bs13"],
                            None, outs["zs"], TS, HMS, "sh", True)
                else:
                    mlp_job(ins[f"xg{j}"], ins[f"w13_{j}"], ins[f"w2_{j}"],
                            ins[f"b13_{j}"], ins[f"scl{j}"], outs[f"oe{j}"],
                            widths[j], HM, f"e{j}", False)

    nc.compile()
    return nc


def _tile_w13(w1e, w3e, n_hm):
    a = w1e.reshape(n_hm, 128, KO, 128).transpose(0, 3, 2, 1)
    b = w3e.reshape(n_hm, 128, KO, 128).transpose(0, 3, 2, 1)
    cat = np.concatenate([a, b], axis=3)           # [n_hm, 128, KO, 256]
    return _to_mm(cat.reshape(n_hm * 128, KO, 256))


def _tile_w2(w2e, n_hm):
    a = w2e.reshape(4, 512, n_hm, 128).transpose(0, 3, 2, 1)
    return _to_mm(a.reshape(4 * 128, n_hm, 512))   # [4*128, n_hm, 512]


def _tile_b13(b1e, b3e, n_hm):
    return np.ascontiguousarray(np.concatenate(
        [b1e.reshape(n_hm, 128).T, b3e.reshape(n_hm, 128).T],
        axis=1).astype(np.float32))                # [128, 2*n_hm]


def kernel(x, gate_w, gate_b, w1, b1, w2, b2, w3, b3,
           ws1, bs1, ws2, bs2, ws3, bs3):
    x = np.asarray(x, np.float32)
    xf = np.ascontiguousarray(x.reshape(-1, D))
    gate_w = np.asarray(gate_w, np.float32)
    gate_b = np.asarray(gate_b, np.float32)
    w1 = np.asarray(w1, np.float32)
    b1 = np.asarray(b1, np.float32)
    w2 = np.asarray(w2, np.float32)
    b2 = np.asarray(b2, np.float32)
    w3 = np.asarray(w3, np.float32)
    b3 = np.asarray(b3, np.float32)
    ws1 = np.asarray(ws1, np.float32)
    bs1 = np.asarray(bs1, np.float32)
    ws2 = np.asarray(ws2, np.float32)
    bs2 = np.asarray(bs2, np.float32)
    ws3 = np.asarray(ws3, np.float32)
    bs3 = np.asarray(bs3, np.float32)

    cw, toks = _host_gate(xf, gate_w, gate_b)
    counts = np.array([len(t) for t in toks])
    widths, asg = _plan_jobs(counts)

    if widths not in _PROGRAM_CACHE:
        _PROGRAM_CACHE[widths] = _build_program(widths)
    nc = _PROGRAM_CACHE[widths]

    xT3 = np.ascontiguousarray(xf.T.reshape(KO, 128, T))   # [KO, 128, T]

    need = sorted({p[0] for slots in asg for p in slots if p is not None})
    w13t = {e: _tile_w13(w1[e], w3[e], HM) for e in need}
    w2t = {e: _tile_w2(w2[e], HM) for e in need}
    b13t = {e: _tile_b13(b1[e], b3[e], HM) for e in need}
    ws13t = _tile_w13(ws1, ws3, HMS)
    ws2t = _tile_w2(ws2, HMS)
    bs13t = _tile_b13(bs1, bs3, HMS)
    zero_b13 = np.zeros((128, 2 * HM), np.float32)

    in_maps = []
    for c in range(N_CORES):
        m = {}
        for j, w in enumerate(widths):
            piece = asg[c][j]
            xg = np.zeros((128, KO, w), np.float32)
            scl = np.zeros(w, np.float32)
            if piece is None:
                e0 = need[0]
                m[f"w13_{j}"] = w13t[e0]
                m[f"w2_{j}"] = w2t[e0]
                m[f"b13_{j}"] = zero_b13
            else:
                e, s0, n = piece
                tk = toks[e][s0:s0 + n]
                xg[:, :, :n] = xT3[:, :, tk].transpose(1, 0, 2)
                scl[:n] = cw[tk, e]
                m[f"w13_{j}"] = w13t[e]
                m[f"w2_{j}"] = w2t[e]
                m[f"b13_{j}"] = b13t[e]
            m[f"xg{j}"] = _to_mm(xg)
            m[f"scl{j}"] = np.ascontiguousarray(scl.reshape(w // 128, 128).T)
        m["xs"] = _to_mm(xT3[:, :, c * TS:(c + 1) * TS].transpose(1, 0, 2))
        m["ws13"] = ws13t
        m["ws2"] = ws2t
        m["bs13"] = bs13t
        in_maps.append(m)

    res = run_bass_kernel_spmd(nc, in_maps, list(range(N_CORES)))

    # host combine: scatter job outputs + concat shared partials
    y = np.zeros((T, D), np.float32)
    for c in range(N_CORES):
        for j, w in enumerate(widths):
            piece = asg[c][j]
            if piece is None:
                continue
            e, s0, n = piece
            tk = toks[e][s0:s0 + n]
            y[tk] += res.results[c][f"oe{j}"][:n]
            y[tk] += cw[tk, e][:, None] * b2[e][None, :]
        y[c * TS:(c + 1) * TS] += res.results[c]["zs"]
    y += bs2[None, :]
    return y.reshape(x.shape).astype(np.float32)


# revision 14
# speedup vs baseline: 1.0161x; 1.0161x over previous
"""Trainium2 Bass kernel for nn_MoE_32332513804634.

MoE: 16 routed experts (top-6, softmax-then-bias routing) + dense shared
expert, T=4096 tokens, D=2048, H=1408, HS=2816, fp32.

Strategy (8 NeuronCores, SPMD):
  - Host computes the gate (cheap) and per-expert token lists.
  - Expert parallelism as a flat list of variable-width "jobs" (128-token
    granularity, up to 1024 tokens each). All cores run the identical
    job-width schedule; the host binds each (core, job) to an arbitrary
    expert piece, so load balance is a pure host-side packing problem.
    A small schedule search minimizes padded capacity (~3200 tokens/core
    vs the 3072 ideal).
  - Weights are streamed per job in DMA-friendly pre-tiled DRAM layouts
    (per-partition contiguous runs of 8-22KB), loaded once per job and
    reused across the job's 512-token chunks.
  - Shared expert is token-parallel: each core runs its 512 tokens
    through the full HS=2816 hidden dim (22 even 128-subtiles, no pad).
  - bf16 matmuls accumulate in fp32 PSUM; per-token combine weights are
    applied as a per-partition DVE scale on the PSUM->SBUF copy; the
    second-layer biases (cw*b2, bs2) are added on the host.
"""

import sys
import numpy as np

sys.path.insert(0, "/opt/trn_rl_repo")

import concourse.bass as bass  # noqa: E402
import concourse.tile as tile  # noqa: E402
from concourse import bacc, mybir  # noqa: E402
from concourse.bass_utils import run_bass_kernel_spmd  # noqa: E402

T = 4096
D = 2048
H = 1408
E = 16
TOP_K = 6
HS = 2816
N_CORES = 8
HM = H // 128           # 11
KO = D // 128           # 16
HMS = HS // 128         # 22
TS = T // N_CORES       # 512 shared-expert tokens per core
F32 = mybir.dt.float32
BF16 = mybir.dt.bfloat16
MM_DT = BF16

_PROGRAM_CACHE: dict = {}


def _to_mm(a):
    import ml_dtypes
    return np.ascontiguousarray(a).astype(ml_dtypes.bfloat16)


def _host_gate(xf, gate_w, gate_b):
    """Numpy replica of the reference gate. Returns cw [T, E] dense combine
    weights and per-expert token lists (ascending)."""
    scores = xf @ gate_w.T
    m = scores.max(axis=-1, keepdims=True)
    p = np.exp(scores - m, dtype=np.float32)
    probs = p / p.sum(axis=-1, keepdims=True)
    biased = probs + gate_b
    idx = np.argpartition(biased, E - TOP_K, axis=1)[:, E - TOP_K:]
    mask = np.zeros((xf.shape[0], E), dtype=bool)
    mask[np.arange(xf.shape[0])[:, None], idx] = True
    cw = np.where(mask, probs, 0.0).astype(np.float32)
    toks = [np.flatnonzero(mask[:, e]).astype(np.int64) for e in range(E)]
    return cw, toks


def _group_partition(counts, widths):
    """Partition the 8x copies of `widths` cells into one cell-group per
    expert with group capacity >= count (global slack bounded by the
    schedule's total padding). Returns groups[e] = list of cell widths,
    or None. Backtracking over experts in descending count order."""
    distinct = sorted(set(widths), reverse=True)
    avail = {w: 8 * widths.count(w) for w in distinct}
    total_cap = sum(w * n for w, n in avail.items())
    order = np.argsort(-np.asarray(counts))
    slack = total_cap - int(np.sum(counts))
    if slack < 0:
        return None

    def combos(target, limit):
        """Cell-width multisets (from current avail) with sum in
        [target, target+limit], cheapest (least waste, fewest) first."""
        out = []

        def rec(i, acc, cap):
            if target <= cap <= target + limit:
                out.append(list(acc))
                return  # adding more cells only wastes
            if i >= len(distinct) or len(out) >= 400:
                return
            w = distinct[i]
            max_n = min(avail[w], -(-(target - cap) // w))
            for n in range(max_n, -1, -1):
                if cap + n * w > target + limit:
                    continue
                acc.extend([w] * n)
                rec(i + 1, acc, cap + n * w)
                if n:
                    del acc[-n:]

        rec(0, [], 0)
        out.sort(key=lambda g: (sum(g), len(g)))
        return out[:80]

    groups = [None] * E
    budget = [4000]

    def solve(k, slack_left):
        if k == len(order):
            return True
        if budget[0] <= 0:
            return False
        budget[0] -= 1
        e = int(order[k])
        target = int(counts[e])
        for g in combos(target, slack_left):
            for w in g:
                avail[w] -= 1
            groups[e] = g
            if solve(k + 1, slack_left - (sum(g) - target)):
                return True
            for w in g:
                avail[w] += 1
            groups[e] = None
        return False

    if not solve(0, slack):
        return None
    return groups


def _pack(counts, widths):
    """Assign expert pieces to an 8 x len(widths) grid of cells via the
    group-partition solver. Returns assignment[core][j] = (expert, start,
    n) | None, or None if infeasible."""
    groups = _group_partition(counts, widths)
    if groups is None:
        return None
    # free cells per width: list of (core, j)
    free = {}
    for j, w in enumerate(widths):
        for c in range(N_CORES):
            free.setdefault(w, []).append((c, j))
    asg = [[None] * len(widths) for _ in range(N_CORES)]
    for e in range(E):
        g = sorted(groups[e], reverse=True)
        rem = int(counts[e])
        start = 0
        for w in g:
            c, j = free[w].pop()
            n = min(rem, w)
            if n > 0:
                asg[c][j] = (int(e), start, int(n))
            start += n
            rem -= n
        assert rem == 0
    return asg


def _plan_jobs(counts):
    """Pick the per-core job-width schedule (identical across cores) and
    the piece assignment. Minimizes padded token capacity, then job
    count, preferring wider minimum job width (DMA-friendlier)."""
    total = int(np.sum(counts))
    percore_lo = -(-total // N_CORES)
    wvocab = [1024, 896, 768, 640, 512, 384, 256, 128]
    cands = []

    def gen(i, acc, cap, ndist):
        if percore_lo <= cap <= percore_lo + 640 and acc:
            cands.append((cap, len(acc), -min(acc), tuple(acc)))
        if i >= len(wvocab) or cap > percore_lo + 640 or len(acc) >= 9:
            return
        w = wvocab[i]
        for n in range(0, min(8, (percore_lo + 640 - cap) // w) + 1):
            nd = ndist + (1 if n else 0)
            if nd > 3:
                break
            gen(i + 1, acc + [w] * n, cap + n * w, nd)

    gen(0, [], 0, 0)
    cands = sorted(set(cands))
    # second pass with more generous capacity in case nothing tight packs
    loose = [(cap + 8 * 512, njobs + 1, mn, w + (512,))
             for cap, njobs, mn, w in cands]
    for cap, njobs, _, widths in cands + loose:
        asg = _pack(counts, widths)
        if asg is not None:
            order = np.argsort([w for w in widths], kind="stable")[::-1]
            widths_o = tuple(int(widths[i]) for i in order)
            asg_o = [[asg[c][int(i)] for i in order] for c in range(N_CORES)]
            return widths_o, asg_o
    raise RuntimeError("no feasible job schedule found")


def _build_program(widths):
    """Build the SPMD Bass program for the given per-core job widths."""
    nc = bacc.Bacc("TRN2", debug=False, num_devices=N_CORES)

    ins = {}
    outs = {}

    def din(name, shape, dt=MM_DT):
        ins[name] = nc.dram_tensor(name, list(shape), dt, kind="ExternalInput").ap()
        return ins[name]

    def dout(name, shape, dt=F32):
        outs[name] = nc.dram_tensor(name, list(shape), dt, kind="ExternalOutput").ap()
        return outs[name]

    for j, w in enumerate(widths):
        din(f"xg{j}", (128, KO, w))
        din(f"w13_{j}", (HM * 128, KO, 256))
        din(f"w2_{j}", (4 * 128, HM, 512))
        din(f"b13_{j}", (128, 2 * HM), F32)
        din(f"scl{j}", (128, w // 128), F32)
        dout(f"oe{j}", (w, D))
    din("xs", (128, KO, TS))
    din("ws13", (HMS * 128, KO, 256))
    din("ws2", (4 * 128, HMS, 512))
    din("bs13", (128, 2 * HMS), F32)
    dout("zs", (TS, D))

    with tile.TileContext(nc) as tc:
        with (
            tc.tile_pool(name="xpool", bufs=2) as xpool,
            tc.tile_pool(name="hpool", bufs=2) as hpool,
            tc.tile_pool(name="wcol", bufs=3) as wcol,
            tc.tile_pool(name="w2pool", bufs=2) as w2pool,
            tc.tile_pool(name="tmp", bufs=2) as tmp,
            tc.tile_pool(name="opool", bufs=3) as opool,
            tc.tile_pool(name="cpool", bufs=1) as cpool,
            tc.tile_pool(name="pp", bufs=2, space="PSUM") as pp,
        ):
            def mlp_job(xg_ap, w13_ap, w2_ap, b13_ap, scl_ap, out_ap,
                        w, n_hm, tag, scale_one):
                """One job: out = scale * (swiglu(x) @ W2^T), W2 bias on host."""
                b13sb = cpool.tile([128, 2 * n_hm], F32, tag=f"b{tag}")
                nc.sync.dma_start(b13sb[:], b13_ap)
                if not scale_one:
                    sclsb = cpool.tile([128, w // 128], F32, tag=f"s{tag}")
                    nc.sync.dma_start(sclsb[:], scl_ap)
                chunks = []
                o = 0
                while o < w:
                    cw_ = min(512, w - o)
                    chunks.append((o, cw_))
                    o += cw_

                xsb = xpool.tile([128, KO, w], MM_DT, tag="xg")
                # two-half load: first matmuls (ko<8) only wait on half the x
                nc.sync.dma_start(xsb[:, 0:KO // 2, :], xg_ap[:, 0:KO // 2, :])
                nc.sync.dma_start(xsb[:, KO // 2:, :], xg_ap[:, KO // 2:, :])
                hsb = hpool.tile([128, n_hm, w], MM_DT, tag="h")
                for hm in range(n_hm):
                    wsb = wcol.tile([128, KO, 256], MM_DT, tag="w13")
                    nc.sync.dma_start(wsb[:], w13_ap[hm * 128:(hm + 1) * 128])
                    for (c0, cw_) in chunks:
                        ps1 = pp.tile([128, cw_], F32, tag="ph1")
                        for ko in range(KO):
                            nc.tensor.matmul(ps1[:], wsb[:, ko, 0:128],
                                             xsb[:, ko, c0:c0 + cw_],
                                             start=(ko == 0), stop=(ko == KO - 1))
                        ps3 = pp.tile([128, cw_], F32, tag="ph3")
                        for ko in range(KO):
                            nc.tensor.matmul(ps3[:], wsb[:, ko, 128:256],
                                             xsb[:, ko, c0:c0 + cw_],
                                             start=(ko == 0), stop=(ko == KO - 1))
                        h1t = tmp.tile([128, cw_], F32, tag="h1t")
                        nc.scalar.activation(h1t[:], ps1[:],
                                             mybir.ActivationFunctionType.Silu,
                                             bias=b13sb[:, hm:hm + 1])
                        h3t = tmp.tile([128, cw_], F32, tag="h3t")
                        nc.scalar.activation(h3t[:], ps3[:],
                                             mybir.ActivationFunctionType.Identity,
                                             bias=b13sb[:, n_hm + hm:n_hm + hm + 1])
                        nc.vector.tensor_mul(hsb[:, hm, c0:c0 + cw_],
                                             h1t[:], h3t[:])
                for dm in range(4):
                    w2sb = w2pool.tile([128, n_hm, 512], MM_DT, tag="w2s")
                    nc.sync.dma_start(w2sb[:], w2_ap[dm * 128:(dm + 1) * 128])
                    for tch in range(w // 128):
                        ps2 = pp.tile([128, 512], F32, tag="po", bufs=4)
                        for k in range(n_hm):
                            nc.tensor.matmul(ps2[:],
                                             hsb[:, k, tch * 128:(tch + 1) * 128],
                                             w2sb[:, k, :],
                                             start=(k == 0), stop=(k == n_hm - 1))
                        osb = opool.tile([128, 512], F32, tag="osb")
                        if scale_one:
                            nc.vector.tensor_copy(osb[:], ps2[:])
                        else:
                            nc.vector.tensor_scalar_mul(
                                osb[:], ps2[:], sclsb[:, tch:tch + 1])
                        nc.sync.dma_start(
                            out_ap[tch * 128:(tch + 1) * 128,
                                   dm * 512:(dm + 1) * 512],
                            osb[:])

            # order: shared expert first (its heavy weight stream gets the
            # DMA-idle window at kernel start), then routed jobs, largest
            # first
            emit = ["sh"] + list(range(len(widths)))
            for j in emit:
                if j == "sh":
                    mlp_job(ins["xs"], ins["ws13"], ins["ws2"], ins["bs13"],
                            None, outs["zs"], TS, HMS, "sh", True)
                else:
                    mlp_job(ins[f"xg{j}"], ins[f"w13_{j}"], ins[f"w2_{j}"],
                            ins[f"b13_{j}"], ins[f"scl{j}"], outs[f"oe{j}"],
                            widths[j], HM, f"e{j}", False)

    nc.compile()
    return nc


def _tile_w13(w1e, w3e, n_hm):
    a = w1e.reshape(n_hm, 128, KO, 128).transpose(0, 3, 2, 1)
    b = w3e.reshape(n_hm, 128, KO, 128).transpose(0, 3, 2, 1)
    cat = np.concatenate([a, b], axis=3)           # [n_hm, 128, KO, 256]
    return _to_mm(cat.reshape(n_hm * 128, KO, 256))


def _tile_w2(w2e, n_hm):
    a = w2e.reshape(4, 512, n_hm, 128).transpose(0, 3, 2, 1)
    return _to_mm(a.reshape(4 * 128, n_hm, 512))   # [4*128, n_hm, 512]


def _tile_b13(b1e, b3e, n_hm):
    return np.ascontiguousarray(np.concatenate(
        [b1e.reshape(n_hm, 128).T, b3e.reshape(n_hm, 128).T],
        axis=1).astype(np.float32))                # [128, 2*n_hm]


def kernel(x, gate_w, gate_b, w1, b1, w2, b2, w3, b3,
           ws1, bs1, ws2, bs2, ws3, bs3):
    x = np.asarray(x, np.float32)
    xf = np.ascontiguousarray(x.reshape(-1, D))
    gate_w = np.asarray(gate_w, np.float32)
    gate_b = np.asarray(gate_b, np.float32)
    w1 = np.asarray(w1, np.float32)
    b1 = np.asarray(b1, np.float32)
    w2 = np.asarray(w2, np.float32)
    b2 = np.asarray(b2, np.float32)
    w3 = np.asarray(w3, np.float32)
    b3 = np.asarray(b3, np.float32)
    ws1 = np.asarray(ws1, np.float32)
    bs1 = np.asarray(bs1, np.float32)
    ws2 = np.asarray(ws2, np.float32)
    bs2 = np.asarray(bs2, np.float32)
    ws3 = np.asarray(ws3, np.float32)
    bs3 = np.asarray(bs3, np.float32)

    cw, toks = _host_gate(xf, gate_w, gate_b)
    counts = np.array([len(t) for t in toks])
    widths, asg = _plan_jobs(counts)

    if widths not in _PROGRAM_CACHE:
        _PROGRAM_CACHE[widths] = _build_program(widths)
    nc = _PROGRAM_CACHE[widths]

    xT3 = np.ascontiguousarray(xf.T.reshape(KO, 128, T))   # [KO, 128, T]

    need = sorted({p[0] for slots in asg for p in slots if p is not None})
    w13t = {e: _tile_w13(w1[e], w3[e], HM) for e in need}
    w2t = {e: _tile_w2(w2[e], HM) for e in need}
    b13t = {e: _tile_b13(b1[e], b3[e], HM) for e in need}
    ws13t = _tile_w13(ws1, ws3, HMS)
    ws2t = _tile_w2(ws2, HMS)
    bs13t = _tile_b13(bs1, bs3, HMS)
    zero_b13 = np.zeros((128, 2 * HM), np.float32)

    in_maps = []
    for c in range(N_CORES):
        m = {}
        for j, w in enumerate(widths):
            piece = asg[c][j]
            xg = np.zeros((128, KO, w), np.float32)
            scl = np.zeros(w, np.float32)
            if piece is None:
                e0 = need[0]
                m[f"w13_{j}"] = w13t[e0]
                m[f"w2_{j}"] = w2t[e0]
                m[f"b13_{j}"] = zero_b13
            else:
                e, s0, n = piece
                tk = toks[e][s0:s0 + n]
                xg[:, :, :n] = xT3[:, :, tk].transpose(1, 0, 2)
                scl[:n] = cw[tk, e]
                m[f"w13_{j}"] = w13t[e]
                m[f"w2_{j}"] = w2t[e]
                m[f"b13_{j}"] = b13t[e]
            m[f"xg{j}"] = _to_mm(xg)
            m[f"scl{j}"] = np.ascontiguousarray(scl.reshape(w // 128, 128).T)
        m["xs"] = _to_mm(xT3[:, :, c * TS:(c + 1) * TS].transpose(1, 0, 2))
        m["ws13"] = ws13t
        m["ws2"] = ws2t
        m["bs13"] = bs13t
        in_maps.append(m)

    res = run_bass_kernel_spmd(nc, in_maps, list(range(N_CORES)))

    # host combine: scatter job outputs + concat shared partials
    y = np.zeros((T, D), np.float32)
    for c in range(N_CORES):
        for j, w in enumerate(widths):
            piece = asg[c][j]
            if piece is None:
                continue
            e, s0, n = piece
            tk = toks[e][s0:s0 + n]
            y[tk] += res.results[c][f"oe{j}"][:n]
            y[tk] += cw[tk, e][:, None] * b2[e][None, :]
        y[c * TS:(c + 1) * TS] += res.results[c]["zs"]
    y += bs2[None, :]
    return y.reshape(x.shape).astype(np.float32)


# revision 18
# speedup vs baseline: 1.0163x; 1.0002x over previous
"""Trainium2 Bass kernel for nn_MoE_32332513804634.

MoE: 16 routed experts (top-6, softmax-then-bias routing) + dense shared
expert, T=4096 tokens, D=2048, H=1408, HS=2816, fp32.

Strategy (8 NeuronCores, SPMD):
  - Host computes the gate (cheap) and per-expert token lists.
  - Expert parallelism as a flat list of variable-width "jobs" (128-token
    granularity, up to 1024 tokens each). All cores run the identical
    job-width schedule; the host binds each (core, job) to an arbitrary
    expert piece, so load balance is a pure host-side packing problem.
    A small schedule search minimizes padded capacity (~3200 tokens/core
    vs the 3072 ideal).
  - Weights are streamed per job in DMA-friendly pre-tiled DRAM layouts
    (per-partition contiguous runs of 8-22KB), loaded once per job and
    reused across the job's 512-token chunks.
  - Shared expert is token-parallel: each core runs its 512 tokens
    through the full HS=2816 hidden dim (22 even 128-subtiles, no pad).
  - bf16 matmuls accumulate in fp32 PSUM; per-token combine weights are
    applied as a per-partition DVE scale on the PSUM->SBUF copy; the
    second-layer biases (cw*b2, bs2) are added on the host.
"""

import sys
import numpy as np

sys.path.insert(0, "/opt/trn_rl_repo")

import concourse.bass as bass  # noqa: E402
import concourse.tile as tile  # noqa: E402
from concourse import bacc, mybir  # noqa: E402
from concourse.bass_utils import run_bass_kernel_spmd  # noqa: E402

T = 4096
D = 2048
H = 1408
E = 16
TOP_K = 6
HS = 2816
N_CORES = 8
HM = H // 128           # 11
KO = D // 128           # 16
HMS = HS // 128         # 22
TS = T // N_CORES       # 512 shared-expert tokens per core
F32 = mybir.dt.float32
BF16 = mybir.dt.bfloat16
MM_DT = BF16

_PROGRAM_CACHE: dict = {}


def _to_mm(a):
    import ml_dtypes
    return np.ascontiguousarray(a).astype(ml_dtypes.bfloat16)


def _host_gate(xf, gate_w, gate_b):
    """Numpy replica of the reference gate. Returns cw [T, E] dense combine
    weights and per-expert token lists (ascending)."""
    scores = xf @ gate_w.T
    m = scores.max(axis=-1, keepdims=True)
    p = np.exp(scores - m, dtype=np.float32)
    probs = p / p.sum(axis=-1, keepdims=True)
    biased = probs + gate_b
    idx = np.argpartition(biased, E - TOP_K, axis=1)[:, E - TOP_K:]
    mask = np.zeros((xf.shape[0], E), dtype=bool)
    mask[np.arange(xf.shape[0])[:, None], idx] = True
    cw = np.where(mask, probs, 0.0).astype(np.float32)
    toks = [np.flatnonzero(mask[:, e]).astype(np.int64) for e in range(E)]
    return cw, toks


def _group_partition(counts, widths):
    """Partition the 8x copies of `widths` cells into one cell-group per
    expert with group capacity >= count (global slack bounded by the
    schedule's total padding). Returns groups[e] = list of cell widths,
    or None. Backtracking over experts in descending count order."""
    distinct = sorted(set(widths), reverse=True)
    avail = {w: 8 * widths.count(w) for w in distinct}
    total_cap = sum(w * n for w, n in avail.items())
    order = np.argsort(-np.asarray(counts))
    slack = total_cap - int(np.sum(counts))
    if slack < 0:
        return None

    def combos(target, limit):
        """Cell-width multisets (from current avail) with sum in
        [target, target+limit], cheapest (least waste, fewest) first."""
        out = []

        def rec(i, acc, cap):
            if target <= cap <= target + limit:
                out.append(list(acc))
                return  # adding more cells only wastes
            if i >= len(distinct) or len(out) >= 400:
                return
            w = distinct[i]
            max_n = min(avail[w], -(-(target - cap) // w))
            for n in range(max_n, -1, -1):
                if cap + n * w > target + limit:
                    continue
                acc.extend([w] * n)
                rec(i + 1, acc, cap + n * w)
                if n:
                    del acc[-n:]

        rec(0, [], 0)
        out.sort(key=lambda g: (sum(g), len(g)))
        return out[:80]

    groups = [None] * E
    budget = [4000]

    def solve(k, slack_left):
        if k == len(order):
            return True
        if budget[0] <= 0:
            return False
        budget[0] -= 1
        e = int(order[k])
        target = int(counts[e])
        for g in combos(target, slack_left):
            for w in g:
                avail[w] -= 1
            groups[e] = g
            if solve(k + 1, slack_left - (sum(g) - target)):
                return True
            for w in g:
                avail[w] += 1
            groups[e] = None
        return False

    if not solve(0, slack):
        return None
    return groups


def _pack(counts, widths):
    """Assign expert pieces to an 8 x len(widths) grid of cells via the
    group-partition solver. Returns assignment[core][j] = (expert, start,
    n) | None, or None if infeasible."""
    groups = _group_partition(counts, widths)
    if groups is None:
        return None
    # free cells per width: list of (core, j)
    free = {}
    for j, w in enumerate(widths):
        for c in range(N_CORES):
            free.setdefault(w, []).append((c, j))
    asg = [[None] * len(widths) for _ in range(N_CORES)]
    for e in range(E):
        g = sorted(groups[e], reverse=True)
        rem = int(counts[e])
        start = 0
        for w in g:
            c, j = free[w].pop()
            n = min(rem, w)
            if n > 0:
                asg[c][j] = (int(e), start, int(n))
            start += n
            rem -= n
        assert rem == 0
    return asg


def _plan_jobs(counts):
    """Pick the per-core job-width schedule (identical across cores) and
    the piece assignment. Minimizes padded token capacity, then job
    count, preferring wider minimum job width (DMA-friendlier)."""
    total = int(np.sum(counts))
    percore_lo = -(-total // N_CORES)
    wvocab = [1024, 896, 768, 640, 512, 384, 256, 128]
    cands = []

    def gen(i, acc, cap, ndist):
        if percore_lo <= cap <= percore_lo + 640 and acc:
            cands.append((cap, len(acc), -min(acc), tuple(acc)))
        if i >= len(wvocab) or cap > percore_lo + 640 or len(acc) >= 9:
            return
        w = wvocab[i]
        for n in range(0, min(8, (percore_lo + 640 - cap) // w) + 1):
            nd = ndist + (1 if n else 0)
            if nd > 3:
                break
            gen(i + 1, acc + [w] * n, cap + n * w, nd)

    gen(0, [], 0, 0)
    cands = sorted(set(cands))
    # second pass with more generous capacity in case nothing tight packs
    loose = [(cap + 8 * 512, njobs + 1, mn, w + (512,))
             for cap, njobs, mn, w in cands]
    for cap, njobs, _, widths in cands + loose:
        asg = _pack(counts, widths)
        if asg is not None:
            order = np.argsort([w for w in widths], kind="stable")[::-1]
            widths_o = tuple(int(widths[i]) for i in order)
            asg_o = [[asg[c][int(i)] for i in order] for c in range(N_CORES)]
            return widths_o, asg_o
    raise RuntimeError("no feasible job schedule found")


def _build_program(widths):
    """Build the SPMD Bass program for the given per-core job widths."""
    nc = bacc.Bacc("TRN2", debug=False, num_devices=N_CORES)

    ins = {}
    outs = {}

    def din(name, shape, dt=MM_DT):
        ins[name] = nc.dram_tensor(name, list(shape), dt, kind="ExternalInput").ap()
        return ins[name]

    def dout(name, shape, dt=F32):
        outs[name] = nc.dram_tensor(name, list(shape), dt, kind="ExternalOutput").ap()
        return outs[name]

    for j, w in enumerate(widths):
        din(f"xg{j}", (128, KO, w))
        din(f"w13_{j}", (HM * 128, KO, 256))
        din(f"w2_{j}", (4 * 128, HM, 512))
        din(f"b13_{j}", (128, 2 * HM), F32)
        din(f"scl{j}", (128, w // 128), F32)
        dout(f"oe{j}", (w, D))
    din("xs", (128, KO, TS))
    din("ws13", (HMS * 128, KO, 256))
    din("ws2", (4 * 128, HMS, 512))
    din("bs13", (128, 2 * HMS), F32)
    dout("zs", (TS, D))

    with tile.TileContext(nc) as tc:
        with (
            tc.tile_pool(name="xpool", bufs=2) as xpool,
            tc.tile_pool(name="hpool", bufs=2) as hpool,
            tc.tile_pool(name="wcol", bufs=3) as wcol,
            tc.tile_pool(name="w2pool", bufs=2) as w2pool,
            tc.tile_pool(name="tmp", bufs=2) as tmp,
            tc.tile_pool(name="opool", bufs=3) as opool,
            tc.tile_pool(name="cpool", bufs=1) as cpool,
            tc.tile_pool(name="pp", bufs=2, space="PSUM") as pp,
        ):
            def mlp_job(xg_ap, w13_ap, w2_ap, b13_ap, scl_ap, out_ap,
                        w, n_hm, tag, scale_one, fine_x=False):
                """One job: out = scale * (swiglu(x) @ W2^T), W2 bias on host."""
                b13sb = cpool.tile([128, 2 * n_hm], F32, tag=f"b{tag}")
                nc.sync.dma_start(b13sb[:], b13_ap)
                if not scale_one:
                    sclsb = cpool.tile([128, w // 128], F32, tag=f"s{tag}")
                    nc.sync.dma_start(sclsb[:], scl_ap)
                chunks = []
                o = 0
                while o < w:
                    cw_ = min(512, w - o)
                    chunks.append((o, cw_))
                    o += cw_

                xsb = xpool.tile([128, KO, w], MM_DT, tag="xg")
                # split load: early matmuls only wait on a prefix of the x;
                # the first-emitted job loads in quarters to cut the lead-in
                nx = 4 if fine_x else 2
                for q in range(nx):
                    ks = q * KO // nx
                    ke = (q + 1) * KO // nx
                    nc.sync.dma_start(xsb[:, ks:ke, :], xg_ap[:, ks:ke, :])
                hsb = hpool.tile([128, n_hm, w], MM_DT, tag="h")
                for hm in range(n_hm):
                    wsb = wcol.tile([128, KO, 256], MM_DT, tag="w13")
                    if fine_x and hm == 0:
                        nc.sync.dma_start(
                            wsb[:, 0:KO // 2, :],
                            w13_ap[0:128, 0:KO // 2, :])
                        nc.sync.dma_start(
                            wsb[:, KO // 2:, :],
                            w13_ap[0:128, KO // 2:, :])
                    else:
                        nc.sync.dma_start(wsb[:], w13_ap[hm * 128:(hm + 1) * 128])
                    for (c0, cw_) in chunks:
                        ps1 = pp.tile([128, cw_], F32, tag="ph1")
                        for ko in range(KO):
                            nc.tensor.matmul(ps1[:], wsb[:, ko, 0:128],
                                             xsb[:, ko, c0:c0 + cw_],
                                             start=(ko == 0), stop=(ko == KO - 1))
                        ps3 = pp.tile([128, cw_], F32, tag="ph3")
                        for ko in range(KO):
                            nc.tensor.matmul(ps3[:], wsb[:, ko, 128:256],
                                             xsb[:, ko, c0:c0 + cw_],
                                             start=(ko == 0), stop=(ko == KO - 1))
                        h1t = tmp.tile([128, cw_], F32, tag="h1t")
                        nc.scalar.activation(h1t[:], ps1[:],
                                             mybir.ActivationFunctionType.Silu,
                                             bias=b13sb[:, hm:hm + 1])
                        h3t = tmp.tile([128, cw_], F32, tag="h3t")
                        nc.scalar.activation(h3t[:], ps3[:],
                                             mybir.ActivationFunctionType.Identity,
                                             bias=b13sb[:, n_hm + hm:n_hm + hm + 1])
                        nc.vector.tensor_mul(hsb[:, hm, c0:c0 + cw_],
                                             h1t[:], h3t[:])
                for dm in range(4):
                    w2sb = w2pool.tile([128, n_hm, 512], MM_DT, tag="w2s")
                    nc.sync.dma_start(w2sb[:], w2_ap[dm * 128:(dm + 1) * 128])
                    for tch in range(w // 128):
                        ps2 = pp.tile([128, 512], F32, tag="po", bufs=4)
                        for k in range(n_hm):
                            nc.tensor.matmul(ps2[:],
                                             hsb[:, k, tch * 128:(tch + 1) * 128],
                                             w2sb[:, k, :],
                                             start=(k == 0), stop=(k == n_hm - 1))
                        osb = opool.tile([128, 512], F32, tag="osb")
                        if scale_one:
                            nc.vector.tensor_copy(osb[:], ps2[:])
                        else:
                            nc.vector.tensor_scalar_mul(
                                osb[:], ps2[:], sclsb[:, tch:tch + 1])
                        nc.sync.dma_start(
                            out_ap[tch * 128:(tch + 1) * 128,
                                   dm * 512:(dm + 1) * 512],
                            osb[:])

            # order: shared expert first (its heavy weight stream gets the
            # DMA-idle window at kernel start), then routed jobs, largest
            # first
            emit = ["sh"] + list(range(len(widths)))
            for j in emit:
                if j == "sh":
                    mlp_job(ins["xs"], ins["ws13"], ins["ws2"], ins["bs13"],
                            None, outs["zs"], TS, HMS, "sh", True,
                            fine_x=True)
                else:
                    mlp_job(ins[f"xg{j}"], ins[f"w13_{j}"], ins[f"w2_{j}"],
                            ins[f"b13_{j}"], ins[f"scl{j}"], outs[f"oe{j}"],
                            widths[j], HM, f"e{j}", False)

    nc.compile()
    return nc


def _tile_w13(w1e, w3e, n_hm):
    a = w1e.reshape(n_hm, 128, KO, 128).transpose(0, 3, 2, 1)
    b = w3e.reshape(n_hm, 128, KO, 128).transpose(0, 3, 2, 1)
    cat = np.concatenate([a, b], axis=3)           # [n_hm, 128, KO, 256]
    return _to_mm(cat.reshape(n_hm * 128, KO, 256))


def _tile_w2(w2e, n_hm):
    a = w2e.reshape(4, 512, n_hm, 128).transpose(0, 3, 2, 1)
    return _to_mm(a.reshape(4 * 128, n_hm, 512))   # [4*128, n_hm, 512]


def _tile_b13(b1e, b3e, n_hm):
    return np.ascontiguousarray(np.concatenate(
        [b1e.reshape(n_hm, 128).T, b3e.reshape(n_hm, 128).T],
        axis=1).astype(np.float32))                # [128, 2*n_hm]


def kernel(x, gate_w, gate_b, w1, b1, w2, b2, w3, b3,
           ws1, bs1, ws2, bs2, ws3, bs3):
    x = np.asarray(x, np.float32)
    xf = np.ascontiguousarray(x.reshape(-1, D))
    gate_w = np.asarray(gate_w, np.float32)
    gate_b = np.asarray(gate_b, np.float32)
    w1 = np.asarray(w1, np.float32)
    b1 = np.asarray(b1, np.float32)
    w2 = np.asarray(w2, np.float32)
    b2 = np.asarray(b2, np.float32)
    w3 = np.asarray(w3, np.float32)
    b3 = np.asarray(b3, np.float32)
    ws1 = np.asarray(ws1, np.float32)
    bs1 = np.asarray(bs1, np.float32)
    ws2 = np.asarray(ws2, np.float32)
    bs2 = np.asarray(bs2, np.float32)
    ws3 = np.asarray(ws3, np.float32)
    bs3 = np.asarray(bs3, np.float32)

    cw, toks = _host_gate(xf, gate_w, gate_b)
    counts = np.array([len(t) for t in toks])
    widths, asg = _plan_jobs(counts)

    if widths not in _PROGRAM_CACHE:
        _PROGRAM_CACHE[widths] = _build_program(widths)
    nc = _PROGRAM_CACHE[widths]

    xT3 = np.ascontiguousarray(xf.T.reshape(KO, 128, T))   # [KO, 128, T]

    need = sorted({p[0] for slots in asg for p in slots if p is not None})
    w13t = {e: _tile_w13(w1[e], w3[e], HM) for e in need}
    w2t = {e: _tile_w2(w2[e], HM) for e in need}
    b13t = {e: _tile_b13(b1[e], b3[e], HM) for e in need}
    ws13t = _tile_w13(ws1, ws3, HMS)
    ws2t = _tile_w2(ws2, HMS)
    bs13t = _tile_b13(bs1, bs3, HMS)
    zero_b13 = np.zeros((128, 2 * HM), np.float32)

    in_maps = []
    for c in range(N_CORES):
        m = {}
        for j, w in enumerate(widths):
            piece = asg[c][j]
            xg = np.zeros((128, KO, w), np.float32)
            scl = np.zeros(w, np.float32)
            if piece is None:
                e0 = need[0]
                m[f"w13_{j}"] = w13t[e0]
                m[f"w2_{j}"] = w2t[e0]
                m[f"b13_{j}"] = zero_b13
            else:
                e, s0, n = piece
                tk = toks[e][s0:s0 + n]
                xg[:, :, :n] = xT3[:, :, tk].transpose(1, 0, 2)
                scl[:n] = cw[tk, e]
                m[f"w13_{j}"] = w13t[e]
                m[f"w2_{j}"] = w2t[e]
                m[f"b13_{j}"] = b13t[e]
            m[f"xg{j}"] = _to_mm(xg)
            m[f"scl{j}"] = np.ascontiguousarray(scl.reshape(w // 128, 128).T)
        m["xs"] = _to_mm(xT3[:, :, c * TS:(c + 1) * TS].transpose(1, 0, 2))
        m["ws13"] = ws13t
        m["ws2"] = ws2t
        m["bs13"] = bs13t
        in_maps.append(m)

    res = run_bass_kernel_spmd(nc, in_maps, list(range(N_CORES)))

    # host combine: scatter job outputs + concat shared partials
    y = np.zeros((T, D), np.float32)
    for c in range(N_CORES):
        for j, w in enumerate(widths):
            piece = asg[c][j]
            if piece is None:
                continue
            e, s0, n = piece
            tk = toks[e][s0:s0 + n]
            y[tk] += res.results[c][f"oe{j}"][:n]
            y[tk] += cw[tk, e][:, None] * b2[e][None, :]
        y[c * TS:(c + 1) * TS] += res.results[c]["zs"]
    y += bs2[None, :]
    return y.reshape(x.shape).astype(np.float32)
